# revision 24
# baseline (speedup 1.0000x reference)
"""BiLSTM-CRF loss kernel for 8 Trainium2 NeuronCores (data-parallel over batch).

Self-contained: hardcodes all shapes from the problem spec.
Returns scalar f32 loss (mean over batch of CRF NLL).

Math reformulation (validated vs reference):
 - LSTM gates via one tanh (sigmoid(x) = 0.5 tanh(x/2) + 0.5); i,f,o weight
   rows pre-halved on host. States kept as c' = 2c, h' = 2h (weights absorb).
   All elementwise cell math in bf16 (tolerance is 2e-2; measured ~1e-4).
 - Reverse-direction masking: add -30000 to i,f,o pre-activations at padded
   steps (forces sigmoids to exactly 0 => state resets).
 - LayerNorm mean-term folded into the feature weights (rank-1 correction
   Wg' = Wg - wsum/200), so feats = rstd*(Wg' h) + c0.  c0 is folded into
   the CRF transition matrix columns (e0 = exp(c0)), so the device exp has
   no bias and the gold-emit part excludes c0.
 - Gold path score: only the emit part sum(fsl*onehot) is computed on
   device; trans_sum + end_term + c0*mask sums are computed on host and
   passed in as `hostpart`.
 - CRF in exp space: w_t = exp(alpha_t) * kappa^t with kappa and e0 folded
   into the transition matrix; bf16 weights.  One matmul per step
   (lhsT=[K,K+1] incl. END-readout row), readout row staged once per 8
   steps.
 - Phases are emission-interleaved: embedding gathers run on the GpSimd
   queue while the LSTM runs; the PE transposes of gathered embeddings and
   the LN/feature chunks (ready middle-out) are emitted between LSTM steps
   so the engine FIFOs stay busy.
"""

import numpy as np
import ml_dtypes

VOCAB, EMBD, HID, K = 100000, 50, 200, 32
H = 100
START, END = 30, 31
B, T = 512, 256
NCORES = 8
BC = B // NCORES            # 64 sequences per core
LN_EPS = 1e-5
KLOG = 4.9                  # -log(kappa)

bf16 = ml_dtypes.bfloat16

_PROGRAM_CACHE = {}
DEBUG_DUMP = False


def _dims(Tn):
    NT = Tn * BC
    return dict(
        NT=NT,
        CH=NT // 512,           # 512-token chunks (8 steps each)
        NQ=4 if Tn >= 32 else 1,
        QT=Tn // (4 if Tn >= 32 else 1),
    )


def _build_program(Tn):
    import concourse.bass as bass
    import concourse.bacc as bacc
    import concourse.mybir as mybir
    import concourse.tile as tile
    from concourse.alu_op_type import AluOpType as op
    from concourse.masks import make_identity
    from contextlib import ExitStack

    dt = mybir.dt
    AF = mybir.ActivationFunctionType
    D = _dims(Tn)
    NT, CH = D["NT"], D["CH"]
    NQ, QT = D["NQ"], D["QT"]
    PKC = NT // 128             # packed free size for LN stats

    nc = bacc.Bacc()

    d_emb = nc.dram_tensor("emb_tab", [VOCAB, EMBD], dt.bfloat16, kind="ExternalInput")
    d_gidx = nc.dram_tensor("gidx", [128, NT // 128], dt.int32, kind="ExternalInput")
    d_wx = {dn: nc.dram_tensor(f"wx_{dn}", [EMBD + 2, 512], dt.bfloat16, kind="ExternalInput")
            for dn in "fb"}
    d_wh = {dn: nc.dram_tensor(f"wh_{dn}", [H, 512], dt.bfloat16, kind="ExternalInput")
            for dn in "fb"}
    d_invm = nc.dram_tensor("invm", [1, NT], dt.bfloat16, kind="ExternalInput")
    d_ones = nc.dram_tensor("ones_row", [1, NT], dt.bfloat16, kind="ExternalInput")
    d_w0 = nc.dram_tensor("w0", [K, BC], dt.bfloat16, kind="ExternalInput")
    d_wgf = nc.dram_tensor("wgt_f", [H, K], dt.bfloat16, kind="ExternalInput")
    d_wgb = nc.dram_tensor("wgt_b", [H, K], dt.bfloat16, kind="ExternalInput")
    d_mmat = nc.dram_tensor("mmat", [K, K + 1], dt.bfloat16, kind="ExternalInput")
    d_ohem = nc.dram_tensor("oh_em", [K, NT], dt.bfloat16, kind="ExternalInput")
    d_ui = nc.dram_tensor("u_idx", [BC, 1], dt.int32, kind="ExternalInput")
    d_lenk = nc.dram_tensor("len_klog", [BC, 1], dt.float32, kind="ExternalInput")
    d_hostpart = nc.dram_tensor("hostpart", [BC, 1], dt.float32, kind="ExternalInput")
    d_loss = nc.dram_tensor("loss", [BC, 1], dt.float32, kind="ExternalOutput")
    if DEBUG_DUMP:
        d_dbg_hf = nc.dram_tensor("dbg_hf", [H, NT], dt.bfloat16, kind="ExternalOutput")
        d_dbg_hb = nc.dram_tensor("dbg_hb", [H, NT], dt.bfloat16, kind="ExternalOutput")
        d_dbg_e = nc.dram_tensor("dbg_e", [K, NT], dt.bfloat16, kind="ExternalOutput")
        d_dbg_rs = nc.dram_tensor("dbg_rs", [1, BC], dt.float32, kind="ExternalOutput")

    with tile.TileContext(nc) as tc, ExitStack() as ctx:
        const = ctx.enter_context(tc.tile_pool(name="const", bufs=1))
        big = ctx.enter_context(tc.tile_pool(name="big", bufs=1))
        dramp = ctx.enter_context(tc.tile_pool(name="dramp", bufs=1, space="DRAM"))

        n_oct = (Tn + 1 + 7) // 8
        u_d = dramp.tile([n_oct * 512, 1], dt.float32, tag="u_d")
        r_d = dramp.tile([BC, 1], dt.float32, tag="r_d")

        ident = const.tile([128, 128], dt.bfloat16)
        make_identity(nc, ident[:])
        wx = {dn: const.tile([EMBD + 2, 512], dt.bfloat16, tag=f"wx{dn}", name=f"wx{dn}") for dn in "fb"}
        wh = {dn: const.tile([H, 512], dt.bfloat16, tag=f"wh{dn}", name=f"wh{dn}") for dn in "fb"}
        for dn in "fb":
            nc.sync.dma_start(wx[dn][:], d_wx[dn][:])
            nc.sync.dma_start(wh[dn][:], d_wh[dn][:])
        wgf = const.tile([H, K], dt.bfloat16)
        wgb = const.tile([H, K], dt.bfloat16)
        nc.sync.dma_start(wgf[:], d_wgf[:])
        nc.sync.dma_start(wgb[:], d_wgb[:])
        mmat = const.tile([K, K + 1], dt.bfloat16)
        nc.sync.dma_start(mmat[:], d_mmat[:])
        ones100 = const.tile([H, 1], dt.bfloat16)
        nc.vector.memset(ones100[:], 1.0)
        ones1kf = const.tile([K, 1], dt.bfloat16)
        nc.vector.memset(ones1kf[:], 1.0)
        ones1k = const.tile([1, K], dt.bfloat16)
        nc.vector.memset(ones1k[:], 1.0)
        gidx = const.tile([128, NT // 128], dt.int32)
        nc.sync.dma_start(gidx[:], d_gidx[:])

        # persistent big tensors
        hq = {dn: [big.tile([H, QT * BC], dt.bfloat16, tag=f"h{dn}{q}", name=f"h{dn}{q}") for q in range(NQ)]
              for dn in "fb"}
        epk = big.tile([K, NT], dt.bfloat16, tag="epk", name="epk")
        xT = big.tile([EMBD + 2, NT], dt.bfloat16, tag="xT", name="xT")
        nc.sync.dma_start(xT[EMBD:EMBD + 1, :], d_ones[:])
        nc.sync.dma_start(xT[EMBD + 1:EMBD + 2, :], d_invm[:])
        mupk = big.tile([128, PKC], dt.float32, tag="mupk")
        msqpk = big.tile([128, PKC], dt.float32, tag="msqpk")
        rstdpk = big.tile([128, PKC], dt.bfloat16, tag="rstdpk")
        epsc = const.tile([128, 1], dt.float32, tag="epsc")
        nc.vector.memset(epsc[:], LN_EPS)

        # ============ pools live for the interleaved main phase =============
        # PSUM budget (8 banks x 2KB): tp 1 + gates 2 + psmu 1 + psmsq 1 +
        # pg 1 + rb 1 + realp 1 = 8.  Main-phase pools close before P3.
        goldps = ctx.enter_context(tc.tile_pool(name="goldps", bufs=1, space="PSUM"))
        realp = goldps.tile([1, 512], dt.float32, tag="realp")

        mctx = ExitStack()
        p0 = mctx.enter_context(tc.tile_pool(name="p0", bufs=4))
        p0ps = mctx.enter_context(tc.tile_pool(name="p0ps", bufs=1, space="PSUM"))
        p1 = mctx.enter_context(tc.tile_pool(name="p1", bufs=3))
        p1s = mctx.enter_context(tc.tile_pool(name="p1s", bufs=1))
        p1ps = mctx.enter_context(tc.tile_pool(name="p1ps", bufs=2, space="PSUM"))
        p2 = mctx.enter_context(tc.tile_pool(name="p2", bufs=3))
        p2ps = mctx.enter_context(tc.tile_pool(name="p2ps", bufs=1, space="PSUM"))
        p2pg = mctx.enter_context(tc.tile_pool(name="p2pg", bufs=1, space="PSUM"))
        p2rb = mctx.enter_context(tc.tile_pool(name="p2rb", bufs=1, space="PSUM"))
        # full-width scratch for the packed-LN math (slices stay
        # partition-aligned with mupk/msqpk rows)
        sqf = big.tile([128, PKC], dt.float32, tag="sqf")
        varf = big.tile([128, PKC], dt.float32, tag="varf")
        lnvf = big.tile([128, PKC], dt.float32, tag="lnvf")

        # ---------------- P0: embedding gather + transpose -----------------
        NB = NT // 512              # token batches of 512 (4 gathers each)
        # gather order: front/back alternating so both LSTM directions are fed
        gorder = []
        for k in range((NB + 1) // 2):
            gorder.append(k)
            if NB - 1 - k > k:
                gorder.append(NB - 1 - k)
        xg_tiles = {}

        def emit_gathers():
            for bi in gorder:
                xg = p0.tile([128, 4 * EMBD], dt.bfloat16, tag="xg")
                for qq in range(4):
                    nc.gpsimd.indirect_dma_start(
                        out=xg[:, qq * EMBD:(qq + 1) * EMBD],
                        out_offset=None,
                        in_=d_emb[:],
                        in_offset=bass.IndirectOffsetOnAxis(
                            ap=gidx[:, bi * 4 + qq:bi * 4 + qq + 1], axis=0),
                    )
                xg_tiles[bi] = xg

        def emit_transpose(bi, use_scalar):
            xg = xg_tiles.pop(bi)
            tp = p0ps.tile([EMBD, 4 * 128], dt.bfloat16, tag="tp")
            for qq in range(4):
                nc.tensor.matmul(
                    out=tp[:, qq * 128:(qq + 1) * 128],
                    lhsT=xg[:, qq * EMBD:(qq + 1) * EMBD],
                    rhs=ident[:], is_transpose=True,
                    start=(qq == 0), stop=(qq == 3),
                )
            dst = bi * 512
            if use_scalar:
                nc.scalar.copy(out=xT[0:EMBD, dst:dst + 512], in_=tp[:])
            else:
                nc.vector.tensor_copy(out=xT[0:EMBD, dst:dst + 512], in_=tp[:])

        # ---------------- P1: LSTM cell (bf16 elementwise) -----------------
        # Emission is staged so the f/b chains pipeline through the engine
        # FIFOs: x-MMs (both dirs) -> h-MMs -> tanh_f, tanh_b -> cell chains
        # (u/t1/C2 per dir, u on GpSimd once the gathers drain) -> thc_f,
        # thc_b -> h_f, h_b.
        cst = {dn: p1s.tile([H, BC], dt.bfloat16, tag=f"c{dn}", name=f"c{dn}") for dn in "fb"}

        def lstm_xmms(dn, pst, col0, t, first):
            rx = xT[:, t * BC:(t + 1) * BC]
            for g in range(4):
                nc.tensor.matmul(out=pst[:, col0 + g * BC:col0 + (g + 1) * BC],
                                 lhsT=wx[dn][:, g * 128:(g + 1) * 128], rhs=rx,
                                 start=True, stop=first)

        def lstm_hmms(dn, pst, col0, prev_t):
            pq, pc = prev_t // QT, (prev_t % QT) * BC
            rh = hq[dn][pq][:, pc:pc + BC]
            for g in range(4):
                nc.tensor.matmul(out=pst[:, col0 + g * BC:col0 + (g + 1) * BC],
                                 lhsT=wh[dn][:, g * 128:(g + 1) * 128], rhs=rh,
                                 start=False, stop=True)

        def lstm_tanh(dn, pst, col0):
            G = p1.tile([H, 4 * BC], dt.bfloat16, tag=f"G{dn}")
            nc.scalar.activation(out=G[:], in_=pst[0:H, col0:col0 + 4 * BC],
                                 func=AF.Tanh)
            return G

        def lstm_cell(dn, G, first, use_gps):
            th_i, th_f, th_g = G[:, 0:BC], G[:, BC:2 * BC], G[:, 3 * BC:4 * BC]
            c = cst[dn]
            if first:
                nc.vector.scalar_tensor_tensor(out=c[:], in0=th_i, scalar=1.0,
                                               in1=th_g, op0=op.add, op1=op.mult)
                return
            u = p1.tile([H, BC], dt.bfloat16, tag=f"u{dn}")
            nc.vector.scalar_tensor_tensor(out=u[:], in0=th_i, scalar=1.0,
                                           in1=th_g, op0=op.add, op1=op.mult)
            t1 = p1.tile([H, BC], dt.bfloat16, tag=f"t1{dn}")
            nc.vector.scalar_tensor_tensor(out=t1[:], in0=th_f, scalar=1.0,
                                           in1=c[:], op0=op.add, op1=op.mult)
            nc.vector.scalar_tensor_tensor(out=c[:], in0=t1[:], scalar=0.5,
                                           in1=u[:], op0=op.mult, op1=op.add)

        def lstm_thc(dn):
            thc = p1.tile([H, BC], dt.bfloat16, tag=f"thc{dn}")
            nc.scalar.activation(out=thc[:], in_=cst[dn][:], func=AF.Tanh, scale=0.5)
            return thc

        def lstm_hout(dn, G, thc, t):
            qh, ch_ = t // QT, (t % QT) * BC
            nc.vector.scalar_tensor_tensor(
                out=hq[dn][qh][:, ch_:ch_ + BC], in0=G[:, 2 * BC:3 * BC],
                scalar=1.0, in1=thc[:], op0=op.add, op1=op.mult)

        # ---------------- P2: LN stats / rstd / feats ----------------------
        # chunk -> packed row slot: symmetric pairs (15-p, 16+p) share a
        # 32-row group so the rstd math stays 32-partition aligned.
        def slot(c):
            if CH == 32:
                return 2 * (15 - c) if c <= 15 else 2 * (c - 16) + 1
            return c

        n_realp = [0]

        def emit_stats(c):
            # column sums of h and h^2 for 512 tokens (both directions)
            psmu = p2ps.tile([1, 512], dt.float32, tag="psmu")
            psmsq = p2ps.tile([1, 512], dt.float32, tag="psmsq")
            q, off = (c * 512) // (QT * BC), (c * 512) % (QT * BC)
            hfc = hq["f"][q][:, off:off + 512]
            hbc = hq["b"][q][:, off:off + 512]
            hsqf = p2.tile([H, 512], dt.bfloat16, tag="hsqf")
            nc.vector.tensor_tensor(out=hsqf[:], in0=hfc, in1=hfc, op=op.mult)
            hsqb = p2.tile([H, 512], dt.bfloat16, tag="hsqb")
            nc.vector.tensor_tensor(out=hsqb[:], in0=hbc, in1=hbc, op=op.mult)
            nc.tensor.matmul(out=psmu[:], lhsT=ones100[:], rhs=hfc,
                             start=True, stop=False)
            nc.tensor.matmul(out=psmu[:], lhsT=ones100[:], rhs=hbc,
                             start=False, stop=True)
            nc.tensor.matmul(out=psmsq[:], lhsT=ones100[:],
                             rhs=hsqf[:], start=True, stop=False)
            nc.tensor.matmul(out=psmsq[:], lhsT=ones100[:],
                             rhs=hsqb[:], start=False, stop=True)
            stgmu = p2.tile([1, 512], dt.float32, tag="stgmu")
            nc.scalar.copy(out=stgmu[:], in_=psmu[:])
            stgmsq = p2.tile([1, 512], dt.float32, tag="stgmsq")
            nc.vector.tensor_copy(out=stgmsq[:], in_=psmsq[:])
            sl = 4 * slot(c)
            nc.sync.dma_start(out=mupk[sl:sl + 4, :], in_=stgmu[:])
            nc.sync.dma_start(out=msqpk[sl:sl + 4, :], in_=stgmsq[:])

        def emit_rstd(r0, r1):
            # rstd rows [r0:r1] of the packed stats (rows 4c..4c+4 per chunk);
            # all operands partition-aligned at rows r0:r1
            nc.vector.scalar_tensor_tensor(
                out=sqf[r0:r1, :], in0=mupk[r0:r1, :], scalar=1.0 / 160000.0,
                in1=mupk[r0:r1, :], op0=op.mult, op1=op.mult)
            nc.vector.scalar_tensor_tensor(
                out=varf[r0:r1, :], in0=msqpk[r0:r1, :], scalar=1.0 / 800.0,
                in1=sqf[r0:r1, :], op0=op.mult, op1=op.subtract)
            nc.scalar.activation(out=lnvf[r0:r1, :], in_=varf[r0:r1, :],
                                 func=AF.Ln, bias=epsc[r0:r1, :])
            nc.scalar.activation(out=rstdpk[r0:r1, :], in_=lnvf[r0:r1, :],
                                 func=AF.Exp, scale=-0.5)

        def emit_feats(c):
            pg = p2pg.tile([K, 512], dt.float32, tag="pg")
            q, off = (c * 512) // (QT * BC), (c * 512) % (QT * BC)
            nc.tensor.matmul(out=pg[:], lhsT=wgf[:], rhs=hq["f"][q][:, off:off + 512],
                             start=True, stop=False)
            nc.tensor.matmul(out=pg[:], lhsT=wgb[:], rhs=hq["b"][q][:, off:off + 512],
                             start=False, stop=True)
            rstg = p2.tile([1, 512], dt.bfloat16, tag="rstg")
            sl = 4 * slot(c)
            nc.sync.dma_start(out=rstg[:], in_=rstdpk[sl:sl + 4, :])
            rb = p2rb.tile([K, 512], dt.float32, tag="rb")
            nc.tensor.matmul(out=rb[:], lhsT=ones1k[:], rhs=rstg[:],
                             start=True, stop=True)
            rbs = p2.tile([K, 512], dt.bfloat16, tag="rbs")
            nc.scalar.copy(out=rbs[:], in_=rb[:])
            fsl = p2.tile([K, 512], dt.bfloat16, tag="fsl")
            nc.vector.tensor_tensor(out=fsl[:], in0=pg[:], in1=rbs[:], op=op.mult)
            nc.scalar.activation(out=epk[:, c * 512:(c + 1) * 512], in_=fsl[:],
                                 func=AF.Exp)
            # gold emit part
            ohem = p2.tile([K, 512], dt.bfloat16, tag="ohem")
            nc.sync.dma_start(ohem[:], d_ohem[:, c * 512:(c + 1) * 512])
            esel = p2.tile([K, 512], dt.bfloat16, tag="esel")
            nc.vector.tensor_tensor(out=esel[:], in0=fsl[:], in1=ohem[:], op=op.mult)
            nc.tensor.matmul(out=realp[:], lhsT=ones1kf[:], rhs=esel[:],
                             start=(n_realp[0] == 0), stop=(n_realp[0] == CH - 1))
            n_realp[0] += 1

        # ---------------- main interleaved emission ------------------------
        emit_gathers()
        # transpose lookahead: batches needed first
        pre = [0, NB - 1, 1, NB - 2]
        for bi in pre:
            emit_transpose(bi, use_scalar=False)
        next_front, next_back = 2, NB - 3

        # P2 chunk pair schedule: pair p (chunks 15-p, 16+p) is ready at
        # iteration 135 + 8p; rstd + feats emitted per group of chunks.
        for s in range(Tn):
            if s % 8 == 0 and next_front <= next_back:
                emit_transpose(next_front, use_scalar=False)
                if next_back > next_front:
                    emit_transpose(next_back, use_scalar=False)
                next_front += 1
                next_back -= 1
            pst = p1ps.tile([128, 8 * BC], dt.float32, tag="g")
            first = (s == 0)
            tf, tb = s, Tn - 1 - s
            lstm_xmms("f", pst, 0, tf, first)
            lstm_xmms("b", pst, 4 * BC, tb, first)
            if not first:
                lstm_hmms("f", pst, 0, tf - 1)
                lstm_hmms("b", pst, 4 * BC, tb + 1)
            Gf = lstm_tanh("f", pst, 0)
            Gb = lstm_tanh("b", pst, 4 * BC)
            use_gps = (120 <= s < 250)
            lstm_cell("f", Gf, first, use_gps)
            lstm_cell("b", Gb, first, use_gps)
            thcf = lstm_thc("f")
            thcb = lstm_thc("b")
            lstm_hout("f", Gf, thcf, tf)
            lstm_hout("b", Gb, thcb, tb)
            if Tn == 256 and s >= 135 and (s - 135) % 8 == 0:
                p = (s - 135) // 8
                if p <= 14:
                    emit_stats(15 - p)
                    emit_stats(16 + p)
                if p == 3:            # chunks 12..19 complete (slots 0..7)
                    emit_rstd(0, 32)
                    for c in range(12, 20):
                        emit_feats(c)
                elif p == 7:          # chunks 8..11, 20..23 (slots 8..15)
                    emit_rstd(32, 64)
                    for c in [8, 9, 10, 11, 20, 21, 22, 23]:
                        emit_feats(c)
                elif p == 11:         # chunks 4..7, 24..27 (slots 16..23)
                    emit_rstd(64, 96)
                    for c in [4, 5, 6, 7, 24, 25, 26, 27]:
                        emit_feats(c)

        if Tn == 256:
            emit_stats(0)
            emit_stats(31)
            emit_rstd(96, 128)
            for c in [0, 1, 2, 3, 28, 29, 30, 31]:
                emit_feats(c)
        else:  # small-T fallback: everything after the loop
            for c in range(CH):
                emit_stats(c)
            emit_rstd(0, 4 * CH)
            for c in range(CH):
                emit_feats(c)
        mctx.close()  # free main-phase PSUM/SBUF pools before the CRF

        if DEBUG_DUMP:
            for q in range(NQ):
                nc.sync.dma_start(d_dbg_hf[:, q * QT * BC:(q + 1) * QT * BC], hq["f"][q][:])
                nc.sync.dma_start(d_dbg_hb[:, q * QT * BC:(q + 1) * QT * BC], hq["b"][q][:])
            nc.sync.dma_start(d_dbg_e[:], epk[:])

        # ---------------- P4 head: gold-score reduce (overlaps the CRF) ----
        with tc.tile_pool(name="p4", bufs=1) as p4, \
             tc.tile_pool(name="p3", bufs=1) as p3, \
             tc.tile_pool(name="p3w", bufs=3) as p3w, \
             tc.tile_pool(name="p3ps", bufs=2, space="PSUM") as p3ps:
            rsub = p4.tile([1, BC], dt.float32, tag="rsub")
            nc.vector.tensor_reduce(
                out=rsub[:], in_=realp[:].rearrange("one (t b) -> one b t", b=BC),
                axis=mybir.AxisListType.X, op=op.add)
            nc.sync.dma_start(out=r_d[:], in_=rsub[:])
            rcol = p4.tile([BC, 1], dt.float32, tag="rcol")
            nc.sync.dma_start(out=rcol[:], in_=r_d[:])
            ui = p4.tile([BC, 1], dt.int32, tag="ui")
            nc.sync.dma_start(ui[:], d_ui[:])
            lenk = p4.tile([BC, 1], dt.float32, tag="lenk")
            nc.sync.dma_start(lenk[:], d_lenk[:])
            hostp = p4.tile([BC, 1], dt.float32, tag="hostp")
            nc.sync.dma_start(hostp[:], d_hostpart[:])

            # ------------ P3: CRF recursion --------------------------------
            w = p3.tile([K, BC], dt.bfloat16, tag="w")
            nc.sync.dma_start(w[:], d_w0[:])
            for o in range(n_oct):
                t0, t1_ = o * 8 + 1, min(o * 8 + 8, Tn + 1)
                pv8 = p3ps.tile([K + 1, 512], dt.float32, tag="pv8")
                for t in range(t0, t1_ + 1):
                    so = (t - 1) % 8
                    ca = so * BC
                    nc.tensor.matmul(out=pv8[:, ca:ca + BC], lhsT=mmat[:],
                                     rhs=w[:], start=True, stop=True)
                    if t <= Tn:
                        tok = (t - 1) * BC
                        wn = p3w.tile([K, BC], dt.bfloat16, tag="wn")
                        nc.vector.tensor_tensor(
                            out=wn[:], in0=pv8[0:K, ca:ca + BC],
                            in1=epk[:, tok:tok + BC], op=op.mult)
                        w = wn
                nsteps = t1_ - t0 + 1
                ustg = p3.tile([1, 512], dt.float32, tag="ustg", bufs=2)
                nc.scalar.copy(out=ustg[:, :nsteps * BC],
                               in_=pv8[K:K + 1, :nsteps * BC])
                nc.sync.dma_start(
                    out=u_d[(t0 - 1) * BC:(t0 - 1) * BC + nsteps * BC, :],
                    in_=ustg[:, :nsteps * BC])

            # ------------ P4 tail: total - real ----------------------------
            ug = p4.tile([BC, 1], dt.float32, tag="ug")
            nc.gpsimd.indirect_dma_start(out=ug[:], out_offset=None, in_=u_d[:],
                                         in_offset=bass.IndirectOffsetOnAxis(ap=ui[:], axis=0))
            tot = p4.tile([BC, 1], dt.float32, tag="tot")
            nc.scalar.activation(out=tot[:], in_=ug[:], func=AF.Ln)
            nc.vector.tensor_tensor(out=tot[:], in0=tot[:], in1=lenk[:], op=op.add)
            if DEBUG_DUMP:
                nc.sync.dma_start(d_dbg_rs[:], rsub[:])
            lout = p4.tile([BC, 1], dt.float32, tag="lout")
            nc.vector.tensor_tensor(out=lout[:], in0=tot[:], in1=rcol[:], op=op.subtract)
            nc.vector.tensor_tensor(out=lout[:], in0=lout[:], in1=hostp[:], op=op.subtract)
            nc.sync.dma_start(out=d_loss[:], in_=lout[:])

    nc.compile()
    return nc


def _prep_core_inputs(sent, tags, slen, consts, Tn):
    """Host-side index prep for one core. sent/tags [BC,Tn] slen [BC]."""
    D = _dims(Tn)
    NT = D["NT"]

    sent_tm = np.ascontiguousarray(sent.T).reshape(-1)      # t-major tokens
    gidx = np.ascontiguousarray(sent_tm.reshape(NT // 128, 128).T).astype(np.int32)

    tgrid = np.repeat(np.arange(Tn), BC)
    bgrid = np.tile(np.arange(BC), Tn)
    invm = (tgrid >= slen[bgrid]).astype(np.float32).reshape(1, NT).astype(bf16)

    tags_ext = np.concatenate([np.full((BC, 1), START, np.int64), tags], axis=1)
    mrow = (tgrid < slen[bgrid]).astype(np.float32)          # [NT] mask, t-major
    tag_tm = tags.T.reshape(-1)                              # tag at token (t,b)
    kk = np.arange(K)[:, None]
    oh_em = ((tag_tm[None, :] == kk) * mrow[None, :]).astype(bf16)

    ui = (slen * BC + np.arange(BC)).astype(np.int32).reshape(BC, 1)

    w0 = np.zeros((K, BC), np.float32)
    w0[START, :] = 1.0

    # host part of the gold score: trans_sum + end_term + c0*mask
    trans, c0 = consts["_trans_f32"], consts["_c0_f32"]
    m = (np.arange(Tn)[None, :] < slen[:, None]).astype(np.float64)
    trans_sum = (trans[tags_ext[:, :Tn], tags_ext[:, 1:]] * m).sum(1)
    end_term = trans[tags_ext[np.arange(BC), slen], END]
    c0_sum = (c0[tags] * m).sum(1)
    hostpart = (trans_sum + end_term + c0_sum).astype(np.float32).reshape(BC, 1)

    d = {k: v for k, v in consts.items() if not k.startswith("_")}
    d.update(dict(
        gidx=gidx,
        invm=invm,
        oh_em=np.ascontiguousarray(oh_em),
        u_idx=ui,
        w0=w0.astype(bf16),
        len_klog=(slen * KLOG).astype(np.float32).reshape(BC, 1),
        hostpart=hostpart,
    ))
    return d


def _pad128(a):
    out = np.zeros((128, a.shape[1]), a.dtype)
    out[:a.shape[0]] = a
    return out


def _prep_consts(emb, Wf_ih, Wf_hh, bfv, Wb_ih, Wb_hh, bbv, gamma, beta, W_lin, trans, Tn):
    D = _dims(Tn)
    sc = np.ones((4 * H, 1), np.float32)
    sc[0:H] = 0.5
    sc[H:2 * H] = 0.5
    sc[3 * H:4 * H] = 0.5
    # reference gate order [i,f,g,o] -> device order [i,f,o,g]
    perm = np.concatenate([np.arange(0, H), np.arange(H, 2 * H),
                           np.arange(3 * H, 4 * H), np.arange(2 * H, 3 * H)])

    def mk(Wi, Wh, b, bwd):
        Wi_s, Wh_s, b_s = Wi * sc, Wh * sc * 0.5, b * sc[:, 0]
        Wi_p, Wh_p, b_p = Wi_s[perm], Wh_s[perm], b_s[perm]
        wxa = np.zeros((EMBD + 2, 512), np.float32)
        wha = np.zeros((H, 512), np.float32)
        for g in range(4):
            wxa[0:EMBD, g * 128:g * 128 + H] = Wi_p.T[:, g * H:(g + 1) * H]
            wxa[EMBD, g * 128:g * 128 + H] = b_p[g * H:(g + 1) * H]
            if bwd and g < 3:
                wxa[EMBD + 1, g * 128:g * 128 + H] = -30000.0  # i,f,o masking
            wha[0:H, g * 128:g * 128 + H] = Wh_p.T[:, g * H:(g + 1) * H]
        return np.ascontiguousarray(wxa).astype(bf16), \
            np.ascontiguousarray(wha).astype(bf16)

    wx_f, wh_f = mk(Wf_ih, Wf_hh, bfv, False)
    wx_b, wh_b = mk(Wb_ih, Wb_hh, bbv, True)

    Wg_full = W_lin * gamma[None, :]
    wsum = Wg_full.sum(1)
    # fold the LN mean subtraction (rank-1) and the h'=2h scaling into Wg
    Wg = (Wg_full - wsum[:, None] / 200.0) * 0.5
    c0 = (W_lin @ beta).astype(np.float64)
    kap = np.exp(-KLOG)
    e0 = np.exp(c0)                       # folded into mmat columns
    mmat = np.zeros((K, K + 1), np.float64)
    mmat[:, :K] = kap * np.exp(trans.astype(np.float64)) * e0[None, :]
    mmat[:, K] = np.exp(trans[:, END].astype(np.float64))

    return dict(
        emb_tab=np.ascontiguousarray(emb).astype(bf16),
        wx_f=wx_f, wh_f=wh_f, wx_b=wx_b, wh_b=wh_b,
        ones_row=np.ones((1, D["NT"]), bf16),
        wgt_f=np.ascontiguousarray(Wg[:, :H].T).astype(bf16),
        wgt_b=np.ascontiguousarray(Wg[:, H:].T).astype(bf16),
        mmat=mmat.astype(bf16),
        _trans_f32=trans.astype(np.float64),
        _c0_f32=c0,
    )


def kernel(sentence, tags, sen_len, emb, Wf_ih, Wf_hh, bf, Wb_ih, Wb_hh, bb,
           gamma, beta, W_lin, trans):
    from concourse import bass_utils

    sentence = np.asarray(sentence).astype(np.int64)
    tags_a = np.asarray(tags).astype(np.int64)
    slen = np.asarray(sen_len).astype(np.int64)
    fp = lambda a: np.ascontiguousarray(np.asarray(a), dtype=np.float32)

    consts = _prep_consts(fp(emb), fp(Wf_ih), fp(Wf_hh), fp(bf), fp(Wb_ih), fp(Wb_hh),
                          fp(bb), fp(gamma), fp(beta), fp(W_lin), fp(trans), T)

    if T not in _PROGRAM_CACHE:
        _PROGRAM_CACHE[T] = _build_program(T)
    nc = _PROGRAM_CACHE[T]

    in_maps = []
    for core in range(NCORES):
        b0 = core * BC
        in_maps.append(_prep_core_inputs(
            sentence[b0:b0 + BC], tags_a[b0:b0 + BC], slen[b0:b0 + BC], consts, T))

    res = bass_utils.run_bass_kernel_spmd(nc, in_maps, core_ids=list(range(NCORES)))
    parts = np.concatenate([r["loss"].reshape(-1) for r in res.results])
    return np.float32(parts.mean())


if __name__ == "__main__":
    import jax
    import reference as R
    cpu = jax.devices("cpu")[0]
    with jax.default_device(cpu):
        inputs = {k: np.asarray(jax.device_put(v, cpu)) for k, v in R.setup_inputs().items()}
        expected = float(R.reference(**{k: jax.device_put(v, cpu) for k, v in inputs.items()}))
    got = kernel(**inputs)
    rel = abs(got - expected) / abs(expected)
    print("expected:", expected, "got:", got, "rel:", rel)
